# revision 8
# baseline (speedup 1.0000x reference)
"""AGNN (2-layer attention GNN) distributed Bass kernel for 8 TRN2 NeuronCores.

Pipeline (per core, dst-sharded):
  h0 = relu(x @ W1 + b1)                      (node shard, x streamed bf16)
  conv1: h1[i] = sum_e softmax_e(cos(h_s, h_d)) * h0[src]
  conv2: same with beta2
  out = h2 @ W2 + b2

Graph prep on host: nodes degree-sorted, dealt round-robin to 8 cores;
per-dst padded slot tables (cross-core-uniform K_t per tile) drive an
indirect-DMA gather from an all-gathered bf16 payload table
[h(32) | invr(1) | pad(1)] per node.  The gather is batched: one
indirect DMA per group of tiles (a few hundred slots), not per slot.

Math restructure vs the naive form:
  logit = beta * (h_s . h_d) * invr_s * invr_d.  invr_d folds into the
  per-partition exp scale; invr_s multiplies the reduced dot.  Padded
  slots point at an all-zero table row, so each pad contributes exactly
  exp(-|beta|) to z; a host-precomputed per-node correction (zadj)
  removes them and adds the self-loop weight, so no mask tensor and no
  segment-max pass are needed (max logit bound == |beta| via self-loop).
"""

import os
import sys
import types

sys.path.insert(0, "/opt/trn_rl_repo")

import numpy as np
import ml_dtypes

import concourse.bass as bass
import concourse.bacc as bacc
import concourse.tile as tile
import concourse.mybir as mybir
from concourse.bass import IndirectOffsetOnAxis
from concourse.bass_utils import run_bass_kernel_spmd
from concourse.masks import make_identity

F32 = mybir.dt.float32
BF16 = mybir.dt.bfloat16
I32 = mybir.dt.int32
AF = mybir.ActivationFunctionType
ALU = mybir.AluOpType
AX = mybir.AxisListType

NC = 8
FIN = 500
FPAD = 512
HID = 32
NCLS = 40
PW = 34          # payload row width (bf16): 32 h + 1 invr + 1 pad
EPS_NORM = 1e-12
EPS_SM = 1e-16
SMAX = 320       # max gathered slots per indirect-DMA group


# ---------------------------------------------------------------- host prep

def _prep_graph(n, edge_index):
    """Degree-sort nodes, deal round-robin to cores, build padded slot tables."""
    src = np.asarray(edge_index[0], dtype=np.int64)
    dst = np.asarray(edge_index[1], dtype=np.int64)
    selfe = src == dst
    mself = np.bincount(dst[selfe], minlength=n)
    keep = ~selfe                               # self-loops handled on-chip
    src, dst = src[keep], dst[keep]
    e = src.shape[0]

    deg = np.bincount(dst, minlength=n)
    order = np.argsort(-deg, kind="stable")          # global rank -> node id
    rank = np.empty(n, dtype=np.int64)
    rank[order] = np.arange(n)

    nsh = -(-n // NC)                                 # nodes per core shard
    tiles = -(-nsh // 128)
    nloc = tiles * 128
    fb = nsh - (tiles - 1) * 128                      # real rows in last tile

    deg_sorted = deg[order]
    ks = []
    for t in range(tiles):
        w0 = t * 128 * NC
        k = int(deg_sorted[w0]) if w0 < n else 1
        ks.append(max(k, 1))
    ks = np.array(ks, dtype=np.int64)
    col0 = np.concatenate([[0], np.cumsum(ks)])       # slot column of tile t
    s_total = int(col0[-1])

    # group tiles: one indirect DMA per group, sum(k) <= SMAX
    groups = []  # (first_tile, ntiles, col_base, slots)
    t = 0
    while t < tiles:
        t0, s = t, 0
        while t < tiles and (t == t0 or s + ks[t] <= SMAX):
            s += ks[t]
            t += 1
        groups.append((t0, t - t0, int(col0[t0]), int(s)))

    # slot column for each edge
    r_dst = rank[dst]
    eorder = np.argsort(r_dst, kind="stable")
    rs = r_dst[eorder]
    ssrc = src[eorder]
    cum = np.concatenate([[0], np.cumsum(deg_sorted)])
    k_in_dst = np.arange(e, dtype=np.int64) - cum[rs]

    core = rs % NC
    local = rs // NC
    t_of = local // 128
    p_of = local % 128

    # table position (payload row) of each node: owner core block + local rank
    tpos = (rank % NC) * nloc + rank // NC
    ztab = nsh                                       # core0's first fake row

    offs = np.full((NC, 128, s_total), ztab, dtype=np.int32)
    offs[core, p_of, col0[t_of] + k_in_dst] = tpos[ssrc].astype(np.int32)

    # per-(core,p,t) degree / self-multiplicity for zadj + self-loop terms
    ranks = np.arange(n)
    c_a, l_a = ranks % NC, ranks // NC
    degarr = np.zeros((NC, 128, tiles), dtype=np.float64)
    degarr[c_a, l_a % 128, l_a // 128] = deg_sorted
    msl = np.zeros((NC, 128, tiles), dtype=np.float64)
    msl[c_a, l_a % 128, l_a // 128] = mself[order]
    padcnt = ks[None, None, :] - degarr              # fake rows: deg 0 -> k

    return dict(order=order, nsh=nsh, tiles=tiles, nloc=nloc, fb=fb,
                ks=[int(k) for k in ks], col0=[int(c) for c in col0],
                s_total=s_total, groups=groups, offs=offs,
                msl=msl, padcnt=padcnt)


# ---------------------------------------------------------------- device build

def _build(tiles, nloc, ks, col0, groups, s_total, stage=99):
    nc = bacc.Bacc("TRN2", target_bir_lowering=False, debug=False,
                   num_devices=NC)

    xT = nc.dram_tensor("xT", [FPAD, nloc], BF16, kind="ExternalInput")
    W1 = nc.dram_tensor("W1", [FPAD, HID], BF16, kind="ExternalInput")
    W2 = nc.dram_tensor("W2", [HID, NCLS], BF16, kind="ExternalInput")
    b2b = nc.dram_tensor("b2b", [128, NCLS], F32, kind="ExternalInput")
    bsc = nc.dram_tensor("bsc", [128, 8], F32, kind="ExternalInput")
    sadj = nc.dram_tensor("sadj", [128, 4 * tiles], F32, kind="ExternalInput")
    offs = nc.dram_tensor("offs", [128 * s_total], I32, kind="ExternalInput")
    out = nc.dram_tensor("out", [nloc, NCLS], F32, kind="ExternalOutput")

    kmax = max(ks)

    with tile.TileContext(nc) as tc:
        with tc.tile_pool(name="const", bufs=1) as cpool, \
             tc.tile_pool(name="persist", bufs=1) as ppool, \
             tc.tile_pool(name="xio", bufs=3) as xpool, \
             tc.tile_pool(name="gio", bufs=6) as gpool, \
             tc.tile_pool(name="work", bufs=6) as wpool, \
             tc.tile_pool(name="small", bufs=8) as spool, \
             tc.tile_pool(name="ps", bufs=2, space="PSUM") as pspool, \
             tc.tile_pool(name="dram", bufs=1, space="DRAM") as dpool:

            # ---- constants to SBUF
            w1_sb = cpool.tile([128, 4, HID], BF16)
            nc.sync.dma_start(w1_sb[:], W1.ap().rearrange("(c p) h -> p c h", p=128))
            w2_sb = cpool.tile([HID, NCLS], BF16)
            nc.sync.dma_start(w2_sb[:], W2.ap())
            b2_sb = cpool.tile([128, NCLS], F32)
            nc.sync.dma_start(b2_sb[:], b2b.ap())
            bsc_sb = cpool.tile([128, 8], F32)
            nc.sync.dma_start(bsc_sb[:], bsc.ap())
            sadj_sb = cpool.tile([128, 4 * tiles], F32)
            nc.sync.dma_start(sadj_sb[:], sadj.ap())
            off_sb = cpool.tile([128, s_total], I32)
            nc.sync.dma_start(off_sb[:],
                              offs.ap().rearrange("(p s) -> p s", s=s_total))
            ident = cpool.tile([128, 128], F32)
            make_identity(nc, ident[:])
            c_eps = cpool.tile([128, 1], F32)
            nc.gpsimd.memset(c_eps[:], EPS_NORM)
            c_neg1 = cpool.tile([128, 1], F32)
            nc.gpsimd.memset(c_neg1[:], -1.0)

            # ---- persistent node state: payload [h | invr | pad] per layer
            pd0 = ppool.tile([128, tiles, PW], BF16)
            pd1 = ppool.tile([128, tiles, PW], BF16)
            inv0 = ppool.tile([128, tiles], F32)      # 1/|h0| per node
            invb = ppool.tile([128, tiles], F32)      # beta2/|h1| per node

            pay1 = dpool.tile([nloc, PW], BF16)
            pay2 = dpool.tile([nloc, PW], BF16)
            tab1 = dpool.tile([NC * nloc, PW], BF16)
            tab2 = dpool.tile([NC * nloc, PW], BF16)

            xT_re = xT.ap().rearrange("(c p) n -> p c n", p=128)

            def payload_finish(pd, inv_ap, pay_dram, t):
                """ss = sum h^2; invr = 1/sqrt(ss+eps) -> inv col + payload."""
                sq = spool.tile([128, HID], BF16, tag="sq")
                ss = spool.tile([128, 1], F32, tag="ss")
                nc.scalar.activation(sq[:], pd[:, t, 0:HID], AF.Square,
                                     accum_out=ss[:])
                rt = spool.tile([128, 1], F32, tag="rt")
                nc.scalar.activation(rt[:], ss[:], AF.Sqrt, bias=c_eps[:])
                nc.vector.reciprocal(inv_ap[:, t:t + 1], rt[:])
                nc.vector.tensor_copy(pd[:, t, HID:HID + 1], inv_ap[:, t:t + 1])
                if t == tiles - 1:
                    # zero payload rows of fake (padding) nodes; bsc[:,2] is
                    # 1 for real partitions of the last tile, 0 for fake
                    nc.vector.tensor_scalar_mul(pd[:, t, :], pd[:, t, :],
                                                bsc_sb[:, 2:3])
                nc.sync.dma_start(pay_dram[t * 128:(t + 1) * 128, :],
                                  pd[:, t, :])

            def dump_h(t, ap32):
                ot = spool.tile([128, NCLS], F32, tag="ot_dbg")
                nc.gpsimd.memset(ot[:], 0.0)
                nc.vector.tensor_copy(ot[:, 0:HID], ap32)
                nc.sync.dma_start(out.ap()[t * 128:(t + 1) * 128, :], ot[:])

            # ================= layer 1 (x @ W1 + b1 via ones-row) + payload
            for t in range(tiles):
                xt = xpool.tile([128, 4, 128], BF16, tag="xt")
                nc.sync.dma_start(xt[:], xT_re[:, :, t * 128:(t + 1) * 128])
                ps = pspool.tile([128, HID], F32, tag="ps1")
                for c in range(4):
                    nc.tensor.matmul(ps[:], lhsT=xt[:, c, :], rhs=w1_sb[:, c, :],
                                     start=(c == 0), stop=(c == 3))
                nc.vector.tensor_relu(pd0[:, t, 0:HID], ps[:])
                if stage == 1:
                    dump_h(t, pd0[:, t, 0:HID])
                    continue
                payload_finish(pd0, inv0, pay1, t)

            if stage >= 2:
                nc.gpsimd.collective_compute(
                    "AllGather", ALU.bypass,
                    replica_groups=[list(range(NC))],
                    ins=[pay1.opt()], outs=[tab1.opt()])

            # ================= conv layer
            def conv(table, pd_src, scale_col, bias_arg, zadj_base, msw_base,
                     epilogue):
                for (t0, nt, cb, sg) in groups:
                    for t in range(t0, t0 + nt):
                        k = ks[t]
                        gbuf = gpool.tile([128, kmax * PW], BF16, tag="g")
                        for kk in range(k):
                            nc.gpsimd.indirect_dma_start(
                                out=gbuf[:, kk * PW:(kk + 1) * PW],
                                out_offset=None, in_=table[:],
                                in_offset=IndirectOffsetOnAxis(
                                    ap=off_sb[:, col0[t] + kk:col0[t] + kk + 1],
                                    axis=0))
                        g3 = gbuf[:, 0:k * PW].rearrange(
                            "p (k d) -> p k d", d=PW)

                        pl = wpool.tile([128, kmax * HID], BF16, tag="pl")
                        pl3 = pl[:, 0:k * HID].rearrange("p (k f) -> p k f",
                                                         f=HID)
                        dbc = pd_src[:, t, 0:HID].unsqueeze(1).to_broadcast(
                            [128, k, HID])
                        nc.vector.tensor_tensor(pl3, g3[:, :, 0:HID], dbc,
                                                ALU.mult)
                        llr = wpool.tile([128, kmax], F32, tag="llr")
                        nc.vector.tensor_reduce(llr[:, 0:k], pl3, AX.X, ALU.add)
                        # ll = llr * invr_src (strided lane-32 read)
                        ll = wpool.tile([128, kmax], F32, tag="ll")
                        nc.vector.tensor_tensor(
                            ll[:, 0:k], llr[:, 0:k],
                            g3[:, :, HID:HID + 1].rearrange("p k d -> p (k d)"),
                            ALU.mult)
                        # w = exp(scale*ll + bias), z = sum(w) fused
                        w = wpool.tile([128, kmax], BF16, tag="w")
                        z = spool.tile([128, 1], F32, tag="z")
                        nc.scalar.activation(w[:, 0:k], ll[:, 0:k], AF.Exp,
                                             bias=bias_arg,
                                             scale=scale_col(t),
                                             accum_out=z[:])
                        z2 = spool.tile([128, 1], F32, tag="z2")
                        nc.vector.tensor_scalar(
                            out=z2[:], in0=z[:],
                            scalar1=sadj_sb[:, zadj_base + t:zadj_base + t + 1],
                            scalar2=float(EPS_SM), op0=ALU.add, op1=ALU.add)
                        rz = spool.tile([128, 1], F32, tag="rz")
                        nc.vector.reciprocal(rz[:], z2[:])

                        pa = wpool.tile([128, kmax * HID], BF16, tag="pa")
                        pa3 = pa[:, 0:k * HID].rearrange("p (k f) -> p k f",
                                                         f=HID)
                        wbc = w[:, 0:k].unsqueeze(2).to_broadcast([128, k, HID])
                        nc.vector.tensor_tensor(pa3, g3[:, :, 0:HID], wbc,
                                                ALU.mult)
                        agg = spool.tile([128, HID], F32, tag="agg")
                        nc.vector.tensor_reduce(
                            agg[:],
                            pa[:, 0:k * HID].rearrange("p (k f) -> p f k",
                                                       f=HID),
                            AX.X, ALU.add)
                        # self-loop: agg += msl*wself * h_dst
                        hw = spool.tile([128, HID], F32, tag="hwself")
                        nc.vector.tensor_scalar_mul(
                            hw[:], pd_src[:, t, 0:HID],
                            sadj_sb[:, msw_base + t:msw_base + t + 1])
                        nc.vector.tensor_add(agg[:], agg[:], hw[:])
                        h_new = spool.tile([128, HID], F32, tag="hnew")
                        nc.vector.tensor_scalar_mul(h_new[:], agg[:], rz[:])
                        epilogue(t, h_new)

            def ep_conv1(t, h_new):
                nc.vector.tensor_copy(pd1[:, t, 0:HID], h_new[:])
                if stage == 2:
                    dump_h(t, h_new[:])
                    return
                payload_finish(pd1, invb, pay2, t)

            if stage >= 2:
                conv(tab1, pd0, lambda t: inv0[:, t:t + 1], c_neg1[:],
                     0, tiles, ep_conv1)

            if stage >= 3:
                # invb currently holds 1/|h1|; scale by beta2 for conv2's exp
                nc.vector.tensor_scalar_mul(invb[:], invb[:], bsc_sb[:, 0:1])

                nc.gpsimd.collective_compute(
                    "AllGather", ALU.bypass,
                    replica_groups=[list(range(NC))],
                    ins=[pay2.opt()], outs=[tab2.opt()])

            def ep_conv2(t, h_new):
                tp = pspool.tile([HID, 128], F32, tag="tp")
                nc.tensor.transpose(out=tp[:], in_=h_new[:], identity=ident[:])
                h2t = spool.tile([HID, 128], BF16, tag="h2t")
                nc.vector.tensor_copy(h2t[:], tp[:])
                ps2 = pspool.tile([128, NCLS], F32, tag="ps2")
                nc.tensor.matmul(ps2[:], lhsT=h2t[:], rhs=w2_sb[:],
                                 start=True, stop=True)
                ot = spool.tile([128, NCLS], F32, tag="ot")
                nc.vector.tensor_tensor(ot[:], ps2[:], b2_sb[:], ALU.add)
                nc.sync.dma_start(out.ap()[t * 128:(t + 1) * 128, :], ot[:])

            if stage >= 3:
                conv(tab2, pd1, lambda t: invb[:, t:t + 1], bsc_sb[:, 1:2],
                     2 * tiles, 3 * tiles, ep_conv2)

    nc.compile()
    return nc


# ---------------------------------------------------------------- entry point

def kernel(x, W1, b1, W2, b2, beta2, edge_index):
    x = np.asarray(x, dtype=np.float32)
    W1 = np.asarray(W1, dtype=np.float32)
    b1 = np.asarray(b1, dtype=np.float32)
    W2 = np.asarray(W2, dtype=np.float32)
    b2 = np.asarray(b2, dtype=np.float32)
    beta2 = float(np.asarray(beta2))
    edge_index = np.asarray(edge_index)

    n = x.shape[0]
    g = _prep_graph(n, edge_index)
    tiles, nloc, nsh, fb = g["tiles"], g["nloc"], g["nsh"], g["fb"]

    nc = _build(tiles, nloc, g["ks"], g["col0"], g["groups"], g["s_total"],
                stage=int(os.environ.get("AGNN_STAGE", "99")))

    w1p = np.zeros((FPAD, HID), dtype=ml_dtypes.bfloat16)
    w1p[:FIN] = W1.astype(ml_dtypes.bfloat16)
    w1p[FIN] = b1.astype(ml_dtypes.bfloat16)
    w2 = W2.astype(ml_dtypes.bfloat16)
    b2b = np.broadcast_to(b2, (128, NCLS)).copy().astype(np.float32)

    bscv = np.zeros((128, 8), dtype=np.float32)
    bscv[:, 0] = beta2
    bscv[:, 1] = -abs(beta2)
    bscv[:fb, 2] = 1.0                          # real-row mask for last tile

    wself1, wself2 = 1.0, float(np.exp(beta2 - abs(beta2)))
    ep1, ep2 = float(np.exp(-1.0)), float(np.exp(-abs(beta2)))
    sadjv = np.zeros((NC, 128, 4 * tiles), dtype=np.float32)
    sadjv[:, :, 0:tiles] = g["msl"] * wself1 - g["padcnt"] * ep1      # zadj1
    sadjv[:, :, tiles:2 * tiles] = g["msl"] * wself1                  # msw1
    sadjv[:, :, 2 * tiles:3 * tiles] = g["msl"] * wself2 - g["padcnt"] * ep2
    sadjv[:, :, 3 * tiles:4 * tiles] = g["msl"] * wself2              # msw2

    order = g["order"]
    xbf = x.astype(ml_dtypes.bfloat16)
    in_maps = []
    for c in range(NC):
        ids = order[c::NC]
        xs = np.zeros((FPAD, nloc), dtype=ml_dtypes.bfloat16)
        xs[:FIN, :len(ids)] = xbf[ids].T
        xs[FIN, :] = 1.0
        in_maps.append({
            "xT": xs, "W1": w1p, "W2": w2, "b2b": b2b, "bsc": bscv,
            "sadj": sadjv[c], "offs": g["offs"][c].reshape(-1),
        })

    trace = os.environ.get("AGNN_TRACE", "") == "1"
    kwargs = {}
    if trace:
        _enable_ntff_hook()
        import tempfile
        base = os.environ.get("AGNN_TRACE_DIR", "/tmp/agnn_traces")
        os.makedirs(base, exist_ok=True)
        kwargs = dict(trace=True, tmpdir=tempfile.mkdtemp(dir=base))
    res = run_bass_kernel_spmd(nc, in_maps, core_ids=list(range(NC)), **kwargs)
    if trace:
        print("AGNN exec_time_ns:", res.exec_time_ns)
        kernel._last_exec_time_ns = res.exec_time_ns

    out_full = np.empty((n, NCLS), dtype=np.float32)
    for c in range(NC):
        ids = order[c::NC]
        out_full[ids] = res.results[c]["out"][:len(ids)]
    return out_full


def _enable_ntff_hook():
    import antenv
    if "antenv.axon_hooks" not in sys.modules:
        mod = types.ModuleType("antenv.axon_hooks")
        _h = [None]
        mod.set_axon_ntff_profile_hook = lambda v: _h.__setitem__(0, v)
        mod.get_axon_ntff_profile_hook = lambda: _h[0]
        sys.modules["antenv.axon_hooks"] = mod
        antenv.axon_hooks = mod
    import concourse.bass_utils as bu
    bu.upload_artifacts = lambda d: d
    from trn_agent_boot.trn_boot import _ntff_profile_via_ctypes
    sys.modules["antenv.axon_hooks"].set_axon_ntff_profile_hook(
        _ntff_profile_via_ctypes("/opt/axon/libaxon_pjrt.so"))


# revision 9
# speedup vs baseline: 1.0091x; 1.0091x over previous
"""AGNN (2-layer attention GNN) distributed Bass kernel for 8 TRN2 NeuronCores.

Pipeline (per core, dst-sharded):
  h0 = relu(x @ W1 + b1)                      (node shard, x streamed bf16)
  conv1: h1[i] = sum_e softmax_e(cos(h_s, h_d)) * h0[src]
  conv2: same with beta2
  out = h2 @ W2 + b2

Graph prep on host: nodes degree-sorted, dealt round-robin to 8 cores;
per-dst padded slot tables (cross-core-uniform K_t per tile) drive an
indirect-DMA gather from an all-gathered bf16 payload table
[h(32) | invr(1) | pad(1)] per node.  The gather is batched: one
indirect DMA per group of tiles (a few hundred slots), not per slot.

Math restructure vs the naive form:
  logit = beta * (h_s . h_d) * invr_s * invr_d.  invr_d folds into the
  per-partition exp scale; invr_s multiplies the reduced dot.  Padded
  slots point at an all-zero table row, so each pad contributes exactly
  exp(-|beta|) to z; a host-precomputed per-node correction (zadj)
  removes them and adds the self-loop weight, so no mask tensor and no
  segment-max pass are needed (max logit bound == |beta| via self-loop).
"""

import os
import sys
import types

sys.path.insert(0, "/opt/trn_rl_repo")

import numpy as np
import ml_dtypes

import concourse.bass as bass
import concourse.bacc as bacc
import concourse.tile as tile
import concourse.mybir as mybir
from concourse.bass import IndirectOffsetOnAxis
from concourse.bass_utils import run_bass_kernel_spmd
from concourse.masks import make_identity

F32 = mybir.dt.float32
BF16 = mybir.dt.bfloat16
I32 = mybir.dt.int32
AF = mybir.ActivationFunctionType
ALU = mybir.AluOpType
AX = mybir.AxisListType

NC = 8
FIN = 500
FPAD = 512
HID = 32
NCLS = 40
PW = 34          # payload row width (bf16): 32 h + 1 invr + 1 pad
EPS_NORM = 1e-12
EPS_SM = 1e-16
SMAX = 320       # max gathered slots per indirect-DMA group


# ---------------------------------------------------------------- host prep

def _prep_graph(n, edge_index):
    """Degree-sort nodes, deal round-robin to cores, build padded slot tables."""
    src = np.asarray(edge_index[0], dtype=np.int64)
    dst = np.asarray(edge_index[1], dtype=np.int64)
    selfe = src == dst
    mself = np.bincount(dst[selfe], minlength=n)
    keep = ~selfe                               # self-loops handled on-chip
    src, dst = src[keep], dst[keep]
    e = src.shape[0]

    deg = np.bincount(dst, minlength=n)
    order = np.argsort(-deg, kind="stable")          # global rank -> node id
    rank = np.empty(n, dtype=np.int64)
    rank[order] = np.arange(n)

    nsh = -(-n // NC)                                 # nodes per core shard
    tiles = -(-nsh // 128)
    nloc = tiles * 128
    fb = nsh - (tiles - 1) * 128                      # real rows in last tile

    deg_sorted = deg[order]
    ks = []
    for t in range(tiles):
        w0 = t * 128 * NC
        k = int(deg_sorted[w0]) if w0 < n else 1
        ks.append(max(k, 1))
    ks = np.array(ks, dtype=np.int64)
    col0 = np.concatenate([[0], np.cumsum(ks)])       # slot column of tile t
    s_total = int(col0[-1])

    # group tiles: one indirect DMA per group, sum(k) <= SMAX
    groups = []  # (first_tile, ntiles, col_base, slots)
    t = 0
    while t < tiles:
        t0, s = t, 0
        while t < tiles and (t == t0 or s + ks[t] <= SMAX):
            s += ks[t]
            t += 1
        groups.append((t0, t - t0, int(col0[t0]), int(s)))

    # slot column for each edge
    r_dst = rank[dst]
    eorder = np.argsort(r_dst, kind="stable")
    rs = r_dst[eorder]
    ssrc = src[eorder]
    cum = np.concatenate([[0], np.cumsum(deg_sorted)])
    k_in_dst = np.arange(e, dtype=np.int64) - cum[rs]

    core = rs % NC
    local = rs // NC
    t_of = local // 128
    p_of = local % 128

    # table position (payload row) of each node: owner core block + local rank
    tpos = (rank % NC) * nloc + rank // NC
    ztab = nsh                                       # core0's first fake row

    offs = np.full((NC, 128, s_total), ztab, dtype=np.int32)
    offs[core, p_of, col0[t_of] + k_in_dst] = tpos[ssrc].astype(np.int32)

    # per-(core,p,t) degree / self-multiplicity for zadj + self-loop terms
    ranks = np.arange(n)
    c_a, l_a = ranks % NC, ranks // NC
    degarr = np.zeros((NC, 128, tiles), dtype=np.float64)
    degarr[c_a, l_a % 128, l_a // 128] = deg_sorted
    msl = np.zeros((NC, 128, tiles), dtype=np.float64)
    msl[c_a, l_a % 128, l_a // 128] = mself[order]
    padcnt = ks[None, None, :] - degarr              # fake rows: deg 0 -> k

    return dict(order=order, nsh=nsh, tiles=tiles, nloc=nloc, fb=fb,
                ks=[int(k) for k in ks], col0=[int(c) for c in col0],
                s_total=s_total, groups=groups, offs=offs,
                msl=msl, padcnt=padcnt)


# ---------------------------------------------------------------- device build

def _build(tiles, nloc, ks, col0, groups, s_total, stage=99):
    nc = bacc.Bacc("TRN2", target_bir_lowering=False, debug=False,
                   num_devices=NC)

    xT = nc.dram_tensor("xT", [FPAD, nloc], BF16, kind="ExternalInput")
    W1 = nc.dram_tensor("W1", [FPAD, HID], BF16, kind="ExternalInput")
    W2 = nc.dram_tensor("W2", [HID, NCLS], BF16, kind="ExternalInput")
    b2b = nc.dram_tensor("b2b", [128, NCLS], F32, kind="ExternalInput")
    bsc = nc.dram_tensor("bsc", [128, 8], F32, kind="ExternalInput")
    sadj = nc.dram_tensor("sadj", [128, 4 * tiles], F32, kind="ExternalInput")
    offs = nc.dram_tensor("offs", [128 * s_total], I32, kind="ExternalInput")
    out = nc.dram_tensor("out", [nloc, NCLS], F32, kind="ExternalOutput")

    kmax = max(ks)

    with tile.TileContext(nc) as tc:
        with tc.tile_pool(name="const", bufs=1) as cpool, \
             tc.tile_pool(name="persist", bufs=1) as ppool, \
             tc.tile_pool(name="xio", bufs=3) as xpool, \
             tc.tile_pool(name="gio", bufs=2) as gpool, \
             tc.tile_pool(name="work", bufs=6) as wpool, \
             tc.tile_pool(name="small", bufs=8) as spool, \
             tc.tile_pool(name="ps", bufs=2, space="PSUM") as pspool, \
             tc.tile_pool(name="dram", bufs=1, space="DRAM") as dpool:

            # ---- constants to SBUF
            w1_sb = cpool.tile([128, 4, HID], BF16)
            nc.sync.dma_start(w1_sb[:], W1.ap().rearrange("(c p) h -> p c h", p=128))
            w2_sb = cpool.tile([HID, NCLS], BF16)
            nc.sync.dma_start(w2_sb[:], W2.ap())
            b2_sb = cpool.tile([128, NCLS], F32)
            nc.sync.dma_start(b2_sb[:], b2b.ap())
            bsc_sb = cpool.tile([128, 8], F32)
            nc.sync.dma_start(bsc_sb[:], bsc.ap())
            sadj_sb = cpool.tile([128, 4 * tiles], F32)
            nc.sync.dma_start(sadj_sb[:], sadj.ap())
            off_sb = cpool.tile([128, s_total], I32)
            nc.sync.dma_start(off_sb[:],
                              offs.ap().rearrange("(p s) -> p s", s=s_total))
            ident = cpool.tile([128, 128], F32)
            make_identity(nc, ident[:])
            c_eps = cpool.tile([128, 1], F32)
            nc.gpsimd.memset(c_eps[:], EPS_NORM)
            c_neg1 = cpool.tile([128, 1], F32)
            nc.gpsimd.memset(c_neg1[:], -1.0)

            # ---- persistent node state: payload [h | invr | pad] per layer
            pd0 = ppool.tile([128, tiles, PW], BF16)
            pd1 = ppool.tile([128, tiles, PW], BF16)
            inv0 = ppool.tile([128, tiles], F32)      # 1/|h0| per node
            invb = ppool.tile([128, tiles], F32)      # beta2/|h1| per node

            pay1 = dpool.tile([nloc, PW], BF16)
            pay2 = dpool.tile([nloc, PW], BF16)
            tab1 = dpool.tile([NC * nloc, PW], BF16)
            tab2 = dpool.tile([NC * nloc, PW], BF16)

            xT_re = xT.ap().rearrange("(c p) n -> p c n", p=128)

            def payload_finish(pd, inv_ap, pay_dram, t):
                """ss = sum h^2; invr = 1/sqrt(ss+eps) -> inv col + payload."""
                sq = spool.tile([128, HID], BF16, tag="sq")
                ss = spool.tile([128, 1], F32, tag="ss")
                nc.scalar.activation(sq[:], pd[:, t, 0:HID], AF.Square,
                                     accum_out=ss[:])
                rt = spool.tile([128, 1], F32, tag="rt")
                nc.scalar.activation(rt[:], ss[:], AF.Sqrt, bias=c_eps[:])
                nc.vector.reciprocal(inv_ap[:, t:t + 1], rt[:])
                nc.vector.tensor_copy(pd[:, t, HID:HID + 1], inv_ap[:, t:t + 1])
                if t == tiles - 1:
                    # zero payload rows of fake (padding) nodes; bsc[:,2] is
                    # 1 for real partitions of the last tile, 0 for fake
                    nc.vector.tensor_scalar_mul(pd[:, t, :], pd[:, t, :],
                                                bsc_sb[:, 2:3])
                nc.sync.dma_start(pay_dram[t * 128:(t + 1) * 128, :],
                                  pd[:, t, :])

            def dump_h(t, ap32):
                ot = spool.tile([128, NCLS], F32, tag="ot_dbg")
                nc.gpsimd.memset(ot[:], 0.0)
                nc.vector.tensor_copy(ot[:, 0:HID], ap32)
                nc.sync.dma_start(out.ap()[t * 128:(t + 1) * 128, :], ot[:])

            # ================= layer 1 (x @ W1 + b1 via ones-row) + payload
            for t in range(tiles):
                xt = xpool.tile([128, 4, 128], BF16, tag="xt")
                nc.sync.dma_start(xt[:], xT_re[:, :, t * 128:(t + 1) * 128])
                ps = pspool.tile([128, HID], F32, tag="ps1")
                for c in range(4):
                    nc.tensor.matmul(ps[:], lhsT=xt[:, c, :], rhs=w1_sb[:, c, :],
                                     start=(c == 0), stop=(c == 3))
                nc.vector.tensor_relu(pd0[:, t, 0:HID], ps[:])
                if stage == 1:
                    dump_h(t, pd0[:, t, 0:HID])
                    continue
                payload_finish(pd0, inv0, pay1, t)

            if stage >= 2:
                nc.gpsimd.collective_compute(
                    "AllGather", ALU.bypass,
                    replica_groups=[list(range(NC))],
                    ins=[pay1.opt()], outs=[tab1.opt()])

            # ================= conv layer
            def conv(table, pd_src, scale_col, bias_arg, zadj_base, msw_base,
                     epilogue):
                for (t0, nt, cb, sg) in groups:
                    for t in range(t0, t0 + nt):
                        k = ks[t]
                        gbuf = gpool.tile([128, kmax * PW], BF16, tag="g")
                        for kk in range(k):
                            nc.gpsimd.indirect_dma_start(
                                out=gbuf[:, kk * PW:(kk + 1) * PW],
                                out_offset=None, in_=table[:],
                                in_offset=IndirectOffsetOnAxis(
                                    ap=off_sb[:, col0[t] + kk:col0[t] + kk + 1],
                                    axis=0))
                        g3 = gbuf[:, 0:k * PW].rearrange(
                            "p (k d) -> p k d", d=PW)

                        pl = wpool.tile([128, kmax * HID], BF16, tag="pl")
                        pl3 = pl[:, 0:k * HID].rearrange("p (k f) -> p k f",
                                                         f=HID)
                        dbc = pd_src[:, t, 0:HID].unsqueeze(1).to_broadcast(
                            [128, k, HID])
                        nc.vector.tensor_tensor(pl3, g3[:, :, 0:HID], dbc,
                                                ALU.mult)
                        llr = wpool.tile([128, kmax], F32, tag="llr")
                        nc.vector.tensor_reduce(llr[:, 0:k], pl3, AX.X, ALU.add)
                        # ll = llr * invr_src (strided lane-32 read)
                        ll = wpool.tile([128, kmax], F32, tag="ll")
                        nc.vector.tensor_tensor(
                            ll[:, 0:k], llr[:, 0:k],
                            g3[:, :, HID:HID + 1].rearrange("p k d -> p (k d)"),
                            ALU.mult)
                        # w = exp(scale*ll + bias), z = sum(w) fused
                        w = wpool.tile([128, kmax], BF16, tag="w")
                        z = spool.tile([128, 1], F32, tag="z")
                        nc.scalar.activation(w[:, 0:k], ll[:, 0:k], AF.Exp,
                                             bias=bias_arg,
                                             scale=scale_col(t),
                                             accum_out=z[:])
                        z2 = spool.tile([128, 1], F32, tag="z2")
                        nc.vector.tensor_scalar(
                            out=z2[:], in0=z[:],
                            scalar1=sadj_sb[:, zadj_base + t:zadj_base + t + 1],
                            scalar2=float(EPS_SM), op0=ALU.add, op1=ALU.add)
                        rz = spool.tile([128, 1], F32, tag="rz")
                        nc.vector.reciprocal(rz[:], z2[:])

                        pa = wpool.tile([128, kmax * HID], BF16, tag="pa")
                        pa3 = pa[:, 0:k * HID].rearrange("p (k f) -> p k f",
                                                         f=HID)
                        wbc = w[:, 0:k].unsqueeze(2).to_broadcast([128, k, HID])
                        nc.vector.tensor_tensor(pa3, g3[:, :, 0:HID], wbc,
                                                ALU.mult)
                        agg = spool.tile([128, HID], F32, tag="agg")
                        nc.vector.tensor_reduce(
                            agg[:],
                            pa[:, 0:k * HID].rearrange("p (k f) -> p f k",
                                                       f=HID),
                            AX.X, ALU.add)
                        # self-loop: agg += msl*wself * h_dst
                        hw = spool.tile([128, HID], F32, tag="hwself")
                        nc.vector.tensor_scalar_mul(
                            hw[:], pd_src[:, t, 0:HID],
                            sadj_sb[:, msw_base + t:msw_base + t + 1])
                        nc.vector.tensor_add(agg[:], agg[:], hw[:])
                        h_new = spool.tile([128, HID], F32, tag="hnew")
                        nc.vector.tensor_scalar_mul(h_new[:], agg[:], rz[:])
                        epilogue(t, h_new)

            def ep_conv1(t, h_new):
                nc.vector.tensor_copy(pd1[:, t, 0:HID], h_new[:])
                if stage == 2:
                    dump_h(t, h_new[:])
                    return
                payload_finish(pd1, invb, pay2, t)

            if stage >= 2:
                conv(tab1, pd0, lambda t: inv0[:, t:t + 1], c_neg1[:],
                     0, tiles, ep_conv1)

            if stage >= 3:
                # invb currently holds 1/|h1|; scale by beta2 for conv2's exp
                nc.vector.tensor_scalar_mul(invb[:], invb[:], bsc_sb[:, 0:1])

                nc.gpsimd.collective_compute(
                    "AllGather", ALU.bypass,
                    replica_groups=[list(range(NC))],
                    ins=[pay2.opt()], outs=[tab2.opt()])

            def ep_conv2(t, h_new):
                tp = pspool.tile([HID, 128], F32, tag="tp")
                nc.tensor.transpose(out=tp[:], in_=h_new[:], identity=ident[:])
                h2t = spool.tile([HID, 128], BF16, tag="h2t")
                nc.vector.tensor_copy(h2t[:], tp[:])
                ps2 = pspool.tile([128, NCLS], F32, tag="ps2")
                nc.tensor.matmul(ps2[:], lhsT=h2t[:], rhs=w2_sb[:],
                                 start=True, stop=True)
                ot = spool.tile([128, NCLS], F32, tag="ot")
                nc.vector.tensor_tensor(ot[:], ps2[:], b2_sb[:], ALU.add)
                nc.sync.dma_start(out.ap()[t * 128:(t + 1) * 128, :], ot[:])

            if stage >= 3:
                conv(tab2, pd1, lambda t: invb[:, t:t + 1], bsc_sb[:, 1:2],
                     2 * tiles, 3 * tiles, ep_conv2)

    nc.compile()
    return nc


# ---------------------------------------------------------------- entry point

def kernel(x, W1, b1, W2, b2, beta2, edge_index):
    x = np.asarray(x, dtype=np.float32)
    W1 = np.asarray(W1, dtype=np.float32)
    b1 = np.asarray(b1, dtype=np.float32)
    W2 = np.asarray(W2, dtype=np.float32)
    b2 = np.asarray(b2, dtype=np.float32)
    beta2 = float(np.asarray(beta2))
    edge_index = np.asarray(edge_index)

    n = x.shape[0]
    g = _prep_graph(n, edge_index)
    tiles, nloc, nsh, fb = g["tiles"], g["nloc"], g["nsh"], g["fb"]

    nc = _build(tiles, nloc, g["ks"], g["col0"], g["groups"], g["s_total"],
                stage=int(os.environ.get("AGNN_STAGE", "99")))

    w1p = np.zeros((FPAD, HID), dtype=ml_dtypes.bfloat16)
    w1p[:FIN] = W1.astype(ml_dtypes.bfloat16)
    w1p[FIN] = b1.astype(ml_dtypes.bfloat16)
    w2 = W2.astype(ml_dtypes.bfloat16)
    b2b = np.broadcast_to(b2, (128, NCLS)).copy().astype(np.float32)

    bscv = np.zeros((128, 8), dtype=np.float32)
    bscv[:, 0] = beta2
    bscv[:, 1] = -abs(beta2)
    bscv[:fb, 2] = 1.0                          # real-row mask for last tile

    wself1, wself2 = 1.0, float(np.exp(beta2 - abs(beta2)))
    ep1, ep2 = float(np.exp(-1.0)), float(np.exp(-abs(beta2)))
    sadjv = np.zeros((NC, 128, 4 * tiles), dtype=np.float32)
    sadjv[:, :, 0:tiles] = g["msl"] * wself1 - g["padcnt"] * ep1      # zadj1
    sadjv[:, :, tiles:2 * tiles] = g["msl"] * wself1                  # msw1
    sadjv[:, :, 2 * tiles:3 * tiles] = g["msl"] * wself2 - g["padcnt"] * ep2
    sadjv[:, :, 3 * tiles:4 * tiles] = g["msl"] * wself2              # msw2

    order = g["order"]
    xbf = x.astype(ml_dtypes.bfloat16)
    in_maps = []
    for c in range(NC):
        ids = order[c::NC]
        xs = np.zeros((FPAD, nloc), dtype=ml_dtypes.bfloat16)
        xs[:FIN, :len(ids)] = xbf[ids].T
        xs[FIN, :] = 1.0
        in_maps.append({
            "xT": xs, "W1": w1p, "W2": w2, "b2b": b2b, "bsc": bscv,
            "sadj": sadjv[c], "offs": g["offs"][c].reshape(-1),
        })

    trace = os.environ.get("AGNN_TRACE", "") == "1"
    kwargs = {}
    if trace:
        _enable_ntff_hook()
        import tempfile
        base = os.environ.get("AGNN_TRACE_DIR", "/tmp/agnn_traces")
        os.makedirs(base, exist_ok=True)
        kwargs = dict(trace=True, tmpdir=tempfile.mkdtemp(dir=base))
    res = run_bass_kernel_spmd(nc, in_maps, core_ids=list(range(NC)), **kwargs)
    if trace:
        print("AGNN exec_time_ns:", res.exec_time_ns)
        kernel._last_exec_time_ns = res.exec_time_ns

    out_full = np.empty((n, NCLS), dtype=np.float32)
    for c in range(NC):
        ids = order[c::NC]
        out_full[ids] = res.results[c]["out"][:len(ids)]
    return out_full


def _enable_ntff_hook():
    import antenv
    if "antenv.axon_hooks" not in sys.modules:
        mod = types.ModuleType("antenv.axon_hooks")
        _h = [None]
        mod.set_axon_ntff_profile_hook = lambda v: _h.__setitem__(0, v)
        mod.get_axon_ntff_profile_hook = lambda: _h[0]
        sys.modules["antenv.axon_hooks"] = mod
        antenv.axon_hooks = mod
    import concourse.bass_utils as bu
    bu.upload_artifacts = lambda d: d
    from trn_agent_boot.trn_boot import _ntff_profile_via_ctypes
    sys.modules["antenv.axon_hooks"].set_axon_ntff_profile_hook(
        _ntff_profile_via_ctypes("/opt/axon/libaxon_pjrt.so"))


# revision 11
# speedup vs baseline: 1.0125x; 1.0033x over previous
"""AGNN (2-layer attention GNN) distributed Bass kernel for 8 TRN2 NeuronCores.

Pipeline (per core, dst-sharded):
  h0 = relu(x @ W1 + b1)                      (node shard, x streamed bf16)
  conv1: h1[i] = sum_e softmax_e(cos(h_s, h_d)) * h0[src]
  conv2: same with beta2
  out = h2 @ W2 + b2

Graph prep on host: nodes degree-sorted, dealt round-robin to 8 cores;
per-dst padded slot tables (cross-core-uniform K_t per tile) drive an
indirect-DMA gather from an all-gathered bf16 payload table
[h(32) | invr(1) | pad(1)] per node.  The gather is batched: one
indirect DMA per group of tiles (a few hundred slots), not per slot.

Math restructure vs the naive form:
  logit = beta * (h_s . h_d) * invr_s * invr_d.  invr_d folds into the
  per-partition exp scale; invr_s multiplies the reduced dot.  Padded
  slots point at an all-zero table row, so each pad contributes exactly
  exp(-|beta|) to z; a host-precomputed per-node correction (zadj)
  removes them and adds the self-loop weight, so no mask tensor and no
  segment-max pass are needed (max logit bound == |beta| via self-loop).
"""

import os
import sys
import types

sys.path.insert(0, "/opt/trn_rl_repo")

import numpy as np
import ml_dtypes

import concourse.bass as bass
import concourse.bacc as bacc
import concourse.tile as tile
import concourse.mybir as mybir
from concourse.bass import IndirectOffsetOnAxis
from concourse.bass_utils import run_bass_kernel_spmd
from concourse.masks import make_identity

F32 = mybir.dt.float32
BF16 = mybir.dt.bfloat16
I32 = mybir.dt.int32
AF = mybir.ActivationFunctionType
ALU = mybir.AluOpType
AX = mybir.AxisListType

NC = 8
FIN = 500
FPAD = 512
HID = 32
NCLS = 40
PW = 34          # payload row width (bf16): 32 h + 1 invr + 1 pad
EPS_NORM = 1e-12
EPS_SM = 1e-16
SMAX = 320       # max gathered slots per indirect-DMA group


# ---------------------------------------------------------------- host prep

def _prep_graph(n, edge_index):
    """Degree-sort nodes, deal round-robin to cores, build padded slot tables."""
    src = np.asarray(edge_index[0], dtype=np.int64)
    dst = np.asarray(edge_index[1], dtype=np.int64)
    selfe = src == dst
    mself = np.bincount(dst[selfe], minlength=n)
    keep = ~selfe                               # self-loops handled on-chip
    src, dst = src[keep], dst[keep]
    e = src.shape[0]

    deg = np.bincount(dst, minlength=n)
    order = np.argsort(-deg, kind="stable")          # global rank -> node id
    rank = np.empty(n, dtype=np.int64)
    rank[order] = np.arange(n)

    nsh = -(-n // NC)                                 # nodes per core shard
    tiles = -(-nsh // 128)
    nloc = tiles * 128
    fb = nsh - (tiles - 1) * 128                      # real rows in last tile

    deg_sorted = deg[order]
    ks = []
    for t in range(tiles):
        w0 = t * 128 * NC
        k = int(deg_sorted[w0]) if w0 < n else 1
        ks.append(max(k, 1))
    ks = np.array(ks, dtype=np.int64)
    col0 = np.concatenate([[0], np.cumsum(ks)])       # slot column of tile t
    s_total = int(col0[-1])

    # group tiles: one indirect DMA per group, sum(k) <= SMAX
    groups = []  # (first_tile, ntiles, col_base, slots)
    t = 0
    while t < tiles:
        t0, s = t, 0
        while t < tiles and (t == t0 or s + ks[t] <= SMAX):
            s += ks[t]
            t += 1
        groups.append((t0, t - t0, int(col0[t0]), int(s)))

    # slot column for each edge
    r_dst = rank[dst]
    eorder = np.argsort(r_dst, kind="stable")
    rs = r_dst[eorder]
    ssrc = src[eorder]
    cum = np.concatenate([[0], np.cumsum(deg_sorted)])
    k_in_dst = np.arange(e, dtype=np.int64) - cum[rs]

    core = rs % NC
    local = rs // NC
    t_of = local // 128
    p_of = local % 128

    # table position (payload row) of each node: owner core block + local rank
    tpos = (rank % NC) * nloc + rank // NC
    ztab = nsh                                       # core0's first fake row

    offs = np.full((NC, 128, s_total), ztab, dtype=np.int32)
    offs[core, p_of, col0[t_of] + k_in_dst] = tpos[ssrc].astype(np.int32)

    # per-(core,p,t) degree / self-multiplicity for zadj + self-loop terms
    ranks = np.arange(n)
    c_a, l_a = ranks % NC, ranks // NC
    degarr = np.zeros((NC, 128, tiles), dtype=np.float64)
    degarr[c_a, l_a % 128, l_a // 128] = deg_sorted
    msl = np.zeros((NC, 128, tiles), dtype=np.float64)
    msl[c_a, l_a % 128, l_a // 128] = mself[order]
    padcnt = ks[None, None, :] - degarr              # fake rows: deg 0 -> k

    return dict(order=order, nsh=nsh, tiles=tiles, nloc=nloc, fb=fb,
                ks=[int(k) for k in ks], col0=[int(c) for c in col0],
                s_total=s_total, groups=groups, offs=offs,
                msl=msl, padcnt=padcnt)


# ---------------------------------------------------------------- device build

def _build(tiles, nloc, ks, col0, groups, s_total, stage=99):
    nc = bacc.Bacc("TRN2", target_bir_lowering=False, debug=False,
                   num_devices=NC)

    xT = nc.dram_tensor("xT", [FPAD, nloc], BF16, kind="ExternalInput")
    W1 = nc.dram_tensor("W1", [FPAD, HID], BF16, kind="ExternalInput")
    W2 = nc.dram_tensor("W2", [HID, NCLS], BF16, kind="ExternalInput")
    b2b = nc.dram_tensor("b2b", [128, NCLS], F32, kind="ExternalInput")
    bsc = nc.dram_tensor("bsc", [128, 8], F32, kind="ExternalInput")
    sadj = nc.dram_tensor("sadj", [128, 4 * tiles], F32, kind="ExternalInput")
    offs = nc.dram_tensor("offs", [128 * s_total], I32, kind="ExternalInput")
    out = nc.dram_tensor("out", [nloc, NCLS], F32, kind="ExternalOutput")

    kmax = max(ks)

    with tile.TileContext(nc) as tc:
        with tc.tile_pool(name="const", bufs=1) as cpool, \
             tc.tile_pool(name="persist", bufs=1) as ppool, \
             tc.tile_pool(name="xio", bufs=3) as xpool, \
             tc.tile_pool(name="gio", bufs=2) as gpool, \
             tc.tile_pool(name="work", bufs=6) as wpool, \
             tc.tile_pool(name="small", bufs=8) as spool, \
             tc.tile_pool(name="ps", bufs=2, space="PSUM") as pspool, \
             tc.tile_pool(name="dram", bufs=1, space="DRAM") as dpool:

            # ---- constants to SBUF
            w1_sb = cpool.tile([128, 4, HID], BF16)
            nc.sync.dma_start(w1_sb[:], W1.ap().rearrange("(c p) h -> p c h", p=128))
            w2_sb = cpool.tile([HID, NCLS], BF16)
            nc.sync.dma_start(w2_sb[:], W2.ap())
            b2_sb = cpool.tile([128, NCLS], F32)
            nc.sync.dma_start(b2_sb[:], b2b.ap())
            bsc_sb = cpool.tile([128, 8], F32)
            nc.sync.dma_start(bsc_sb[:], bsc.ap())
            sadj_sb = cpool.tile([128, 4 * tiles], F32)
            nc.sync.dma_start(sadj_sb[:], sadj.ap())
            off_sb = cpool.tile([128, s_total], I32)
            nc.sync.dma_start(off_sb[:],
                              offs.ap().rearrange("(p s) -> p s", s=s_total))
            ident = cpool.tile([128, 128], F32)
            make_identity(nc, ident[:])
            c_eps = cpool.tile([128, 1], F32)
            nc.gpsimd.memset(c_eps[:], EPS_NORM)
            c_neg1 = cpool.tile([128, 1], F32)
            nc.gpsimd.memset(c_neg1[:], -1.0)

            # ---- persistent node state: payload [h | invr | pad] per layer
            pd0 = ppool.tile([128, tiles, PW], BF16)
            pd1 = ppool.tile([128, tiles, PW], BF16)
            inv0 = ppool.tile([128, tiles], F32)      # 1/|h0| per node
            invb = ppool.tile([128, tiles], F32)      # beta2/|h1| per node

            pay1 = dpool.tile([nloc, PW], BF16)
            pay2 = dpool.tile([nloc, PW], BF16)
            tab1 = dpool.tile([NC * nloc, PW], BF16)
            tab2 = dpool.tile([NC * nloc, PW], BF16)

            xT_re = xT.ap().rearrange("(c p) n -> p c n", p=128)

            def payload_finish(pd, inv_ap, pay_dram, t):
                """ss = sum h^2; invr = 1/sqrt(ss+eps) -> inv col + payload."""
                sq = spool.tile([128, HID], BF16, tag="sq")
                ss = spool.tile([128, 1], F32, tag="ss")
                nc.scalar.activation(sq[:], pd[:, t, 0:HID], AF.Square,
                                     accum_out=ss[:])
                rt = spool.tile([128, 1], F32, tag="rt")
                nc.scalar.activation(rt[:], ss[:], AF.Sqrt, bias=c_eps[:])
                nc.vector.reciprocal(inv_ap[:, t:t + 1], rt[:])
                nc.vector.tensor_copy(pd[:, t, HID:HID + 1], inv_ap[:, t:t + 1])
                if t == tiles - 1:
                    # zero payload rows of fake (padding) nodes; bsc[:,2] is
                    # 1 for real partitions of the last tile, 0 for fake
                    nc.vector.tensor_scalar_mul(pd[:, t, :], pd[:, t, :],
                                                bsc_sb[:, 2:3])
                if t % 2 == 1:
                    nc.sync.dma_start(
                        pay_dram[(t - 1) * 128:(t + 1) * 128, :]
                        .rearrange("(a p) d -> p a d", p=128),
                        pd[:, t - 1:t + 1, :])
                elif t == tiles - 1:
                    nc.sync.dma_start(pay_dram[t * 128:(t + 1) * 128, :],
                                      pd[:, t, :])

            def dump_h(t, ap32):
                ot = spool.tile([128, NCLS], F32, tag="ot_dbg")
                nc.gpsimd.memset(ot[:], 0.0)
                nc.vector.tensor_copy(ot[:, 0:HID], ap32)
                nc.sync.dma_start(out.ap()[t * 128:(t + 1) * 128, :], ot[:])

            # ================= layer 1 (x @ W1 + b1 via ones-row) + payload
            for t2 in range(0, tiles, 2):
                npair = min(2, tiles - t2)
                xt = xpool.tile([128, 4, 256], BF16, tag="xt")
                nc.sync.dma_start(
                    xt[:, :, 0:npair * 128],
                    xT_re[:, :, t2 * 128:(t2 + npair) * 128])
                for dt_ in range(npair):
                    t = t2 + dt_
                    ps = pspool.tile([128, HID], F32, tag="ps1")
                    for c in range(4):
                        nc.tensor.matmul(ps[:],
                                         lhsT=xt[:, c, dt_ * 128:(dt_ + 1) * 128],
                                         rhs=w1_sb[:, c, :],
                                         start=(c == 0), stop=(c == 3))
                    nc.vector.tensor_relu(pd0[:, t, 0:HID], ps[:])
                    if stage == 1:
                        dump_h(t, pd0[:, t, 0:HID])
                        continue
                    payload_finish(pd0, inv0, pay1, t)

            if stage >= 2:
                nc.gpsimd.collective_compute(
                    "AllGather", ALU.bypass,
                    replica_groups=[list(range(NC))],
                    ins=[pay1.opt()], outs=[tab1.opt()])

            # ================= conv layer
            def conv(table, pd_src, scale_col, bias_arg, zadj_base, msw_base,
                     epilogue):
                for (t0, nt, cb, sg) in groups:
                    for t in range(t0, t0 + nt):
                        k = ks[t]
                        gbuf = gpool.tile([128, kmax * PW], BF16, tag="g")
                        for kk in range(k):
                            nc.gpsimd.indirect_dma_start(
                                out=gbuf[:, kk * PW:(kk + 1) * PW],
                                out_offset=None, in_=table[:],
                                in_offset=IndirectOffsetOnAxis(
                                    ap=off_sb[:, col0[t] + kk:col0[t] + kk + 1],
                                    axis=0))
                        g3 = gbuf[:, 0:k * PW].rearrange(
                            "p (k d) -> p k d", d=PW)

                        pl = wpool.tile([128, kmax * HID], BF16, tag="pl")
                        pl3 = pl[:, 0:k * HID].rearrange("p (k f) -> p k f",
                                                         f=HID)
                        dbc = pd_src[:, t, 0:HID].unsqueeze(1).to_broadcast(
                            [128, k, HID])
                        nc.vector.tensor_tensor(pl3, g3[:, :, 0:HID], dbc,
                                                ALU.mult)
                        llr = wpool.tile([128, kmax], F32, tag="llr")
                        nc.vector.tensor_reduce(llr[:, 0:k], pl3, AX.X, ALU.add)
                        # ll = llr * invr_src (strided lane-32 read)
                        ll = wpool.tile([128, kmax], F32, tag="ll")
                        nc.vector.tensor_tensor(
                            ll[:, 0:k], llr[:, 0:k],
                            g3[:, :, HID:HID + 1].rearrange("p k d -> p (k d)"),
                            ALU.mult)
                        # w = exp(scale*ll + bias), z = sum(w) fused
                        w = wpool.tile([128, kmax], BF16, tag="w")
                        z = spool.tile([128, 1], F32, tag="z")
                        nc.scalar.activation(w[:, 0:k], ll[:, 0:k], AF.Exp,
                                             bias=bias_arg,
                                             scale=scale_col(t),
                                             accum_out=z[:])
                        z2 = spool.tile([128, 1], F32, tag="z2")
                        nc.vector.tensor_scalar(
                            out=z2[:], in0=z[:],
                            scalar1=sadj_sb[:, zadj_base + t:zadj_base + t + 1],
                            scalar2=float(EPS_SM), op0=ALU.add, op1=ALU.add)
                        rz = spool.tile([128, 1], F32, tag="rz")
                        nc.vector.reciprocal(rz[:], z2[:])

                        pa = wpool.tile([128, kmax * HID], BF16, tag="pa")
                        pa3 = pa[:, 0:k * HID].rearrange("p (k f) -> p k f",
                                                         f=HID)
                        wbc = w[:, 0:k].unsqueeze(2).to_broadcast([128, k, HID])
                        nc.vector.tensor_tensor(pa3, g3[:, :, 0:HID], wbc,
                                                ALU.mult)
                        agg = spool.tile([128, HID], F32, tag="agg")
                        nc.vector.tensor_reduce(
                            agg[:],
                            pa[:, 0:k * HID].rearrange("p (k f) -> p f k",
                                                       f=HID),
                            AX.X, ALU.add)
                        # self-loop: agg += msl*wself * h_dst
                        hw = spool.tile([128, HID], F32, tag="hwself")
                        nc.vector.tensor_scalar_mul(
                            hw[:], pd_src[:, t, 0:HID],
                            sadj_sb[:, msw_base + t:msw_base + t + 1])
                        nc.vector.tensor_add(agg[:], agg[:], hw[:])
                        h_new = spool.tile([128, HID], F32, tag="hnew")
                        nc.vector.tensor_scalar_mul(h_new[:], agg[:], rz[:])
                        epilogue(t, h_new)

            def ep_conv1(t, h_new):
                nc.vector.tensor_copy(pd1[:, t, 0:HID], h_new[:])
                if stage == 2:
                    dump_h(t, h_new[:])
                    return
                payload_finish(pd1, invb, pay2, t)

            if stage >= 2:
                conv(tab1, pd0, lambda t: inv0[:, t:t + 1], c_neg1[:],
                     0, tiles, ep_conv1)

            if stage >= 3:
                # invb currently holds 1/|h1|; scale by beta2 for conv2's exp
                nc.vector.tensor_scalar_mul(invb[:], invb[:], bsc_sb[:, 0:1])

                nc.gpsimd.collective_compute(
                    "AllGather", ALU.bypass,
                    replica_groups=[list(range(NC))],
                    ins=[pay2.opt()], outs=[tab2.opt()])

            def ep_conv2(t, h_new):
                tp = pspool.tile([HID, 128], F32, tag="tp")
                nc.tensor.transpose(out=tp[:], in_=h_new[:], identity=ident[:])
                h2t = spool.tile([HID, 128], BF16, tag="h2t")
                nc.vector.tensor_copy(h2t[:], tp[:])
                ps2 = pspool.tile([128, NCLS], F32, tag="ps2")
                nc.tensor.matmul(ps2[:], lhsT=h2t[:], rhs=w2_sb[:],
                                 start=True, stop=True)
                ot = spool.tile([128, NCLS], F32, tag="ot")
                nc.vector.tensor_tensor(ot[:], ps2[:], b2_sb[:], ALU.add)
                nc.sync.dma_start(out.ap()[t * 128:(t + 1) * 128, :], ot[:])

            if stage >= 3:
                conv(tab2, pd1, lambda t: invb[:, t:t + 1], bsc_sb[:, 1:2],
                     2 * tiles, 3 * tiles, ep_conv2)

    nc.compile()
    return nc


# ---------------------------------------------------------------- entry point

def kernel(x, W1, b1, W2, b2, beta2, edge_index):
    x = np.asarray(x, dtype=np.float32)
    W1 = np.asarray(W1, dtype=np.float32)
    b1 = np.asarray(b1, dtype=np.float32)
    W2 = np.asarray(W2, dtype=np.float32)
    b2 = np.asarray(b2, dtype=np.float32)
    beta2 = float(np.asarray(beta2))
    edge_index = np.asarray(edge_index)

    n = x.shape[0]
    g = _prep_graph(n, edge_index)
    tiles, nloc, nsh, fb = g["tiles"], g["nloc"], g["nsh"], g["fb"]

    nc = _build(tiles, nloc, g["ks"], g["col0"], g["groups"], g["s_total"],
                stage=int(os.environ.get("AGNN_STAGE", "99")))

    w1p = np.zeros((FPAD, HID), dtype=ml_dtypes.bfloat16)
    w1p[:FIN] = W1.astype(ml_dtypes.bfloat16)
    w1p[FIN] = b1.astype(ml_dtypes.bfloat16)
    w2 = W2.astype(ml_dtypes.bfloat16)
    b2b = np.broadcast_to(b2, (128, NCLS)).copy().astype(np.float32)

    bscv = np.zeros((128, 8), dtype=np.float32)
    bscv[:, 0] = beta2
    bscv[:, 1] = -abs(beta2)
    bscv[:fb, 2] = 1.0                          # real-row mask for last tile

    wself1, wself2 = 1.0, float(np.exp(beta2 - abs(beta2)))
    ep1, ep2 = float(np.exp(-1.0)), float(np.exp(-abs(beta2)))
    sadjv = np.zeros((NC, 128, 4 * tiles), dtype=np.float32)
    sadjv[:, :, 0:tiles] = g["msl"] * wself1 - g["padcnt"] * ep1      # zadj1
    sadjv[:, :, tiles:2 * tiles] = g["msl"] * wself1                  # msw1
    sadjv[:, :, 2 * tiles:3 * tiles] = g["msl"] * wself2 - g["padcnt"] * ep2
    sadjv[:, :, 3 * tiles:4 * tiles] = g["msl"] * wself2              # msw2

    order = g["order"]
    xbf = x.astype(ml_dtypes.bfloat16)
    in_maps = []
    for c in range(NC):
        ids = order[c::NC]
        xs = np.zeros((FPAD, nloc), dtype=ml_dtypes.bfloat16)
        xs[:FIN, :len(ids)] = xbf[ids].T
        xs[FIN, :] = 1.0
        in_maps.append({
            "xT": xs, "W1": w1p, "W2": w2, "b2b": b2b, "bsc": bscv,
            "sadj": sadjv[c], "offs": g["offs"][c].reshape(-1),
        })

    trace = os.environ.get("AGNN_TRACE", "") == "1"
    kwargs = {}
    if trace:
        _enable_ntff_hook()
        import tempfile
        base = os.environ.get("AGNN_TRACE_DIR", "/tmp/agnn_traces")
        os.makedirs(base, exist_ok=True)
        kwargs = dict(trace=True, tmpdir=tempfile.mkdtemp(dir=base))
    res = run_bass_kernel_spmd(nc, in_maps, core_ids=list(range(NC)), **kwargs)
    if trace:
        print("AGNN exec_time_ns:", res.exec_time_ns)
        kernel._last_exec_time_ns = res.exec_time_ns

    out_full = np.empty((n, NCLS), dtype=np.float32)
    for c in range(NC):
        ids = order[c::NC]
        out_full[ids] = res.results[c]["out"][:len(ids)]
    return out_full


def _enable_ntff_hook():
    import antenv
    if "antenv.axon_hooks" not in sys.modules:
        mod = types.ModuleType("antenv.axon_hooks")
        _h = [None]
        mod.set_axon_ntff_profile_hook = lambda v: _h.__setitem__(0, v)
        mod.get_axon_ntff_profile_hook = lambda: _h[0]
        sys.modules["antenv.axon_hooks"] = mod
        antenv.axon_hooks = mod
    import concourse.bass_utils as bu
    bu.upload_artifacts = lambda d: d
    from trn_agent_boot.trn_boot import _ntff_profile_via_ctypes
    sys.modules["antenv.axon_hooks"].set_axon_ntff_profile_hook(
        _ntff_profile_via_ctypes("/opt/axon/libaxon_pjrt.so"))


# revision 12
# speedup vs baseline: 1.0184x; 1.0059x over previous
"""AGNN (2-layer attention GNN) distributed Bass kernel for 8 TRN2 NeuronCores.

Pipeline (per core, dst-sharded):
  h0 = relu(x @ W1 + b1)                      (node shard, x streamed bf16)
  conv1: h1[i] = sum_e softmax_e(cos(h_s, h_d)) * h0[src]
  conv2: same with beta2
  out = h2 @ W2 + b2

Graph prep on host: nodes degree-sorted, dealt round-robin to 8 cores;
per-dst padded slot tables (cross-core-uniform K_t per tile) drive an
indirect-DMA gather from an all-gathered bf16 payload table
[h(32) | invr(1) | pad(1)] per node.  The gather is batched: one
indirect DMA per group of tiles (a few hundred slots), not per slot.

Math restructure vs the naive form:
  logit = beta * (h_s . h_d) * invr_s * invr_d.  invr_d folds into the
  per-partition exp scale; invr_s multiplies the reduced dot.  Padded
  slots point at an all-zero table row, so each pad contributes exactly
  exp(-|beta|) to z; a host-precomputed per-node correction (zadj)
  removes them and adds the self-loop weight, so no mask tensor and no
  segment-max pass are needed (max logit bound == |beta| via self-loop).
"""

import os
import sys
import types

sys.path.insert(0, "/opt/trn_rl_repo")

import numpy as np
import ml_dtypes

import concourse.bass as bass
import concourse.bacc as bacc
import concourse.tile as tile
import concourse.mybir as mybir
from concourse.bass import IndirectOffsetOnAxis
from concourse.bass_utils import run_bass_kernel_spmd
from concourse.masks import make_identity

F32 = mybir.dt.float32
BF16 = mybir.dt.bfloat16
I32 = mybir.dt.int32
AF = mybir.ActivationFunctionType
ALU = mybir.AluOpType
AX = mybir.AxisListType

NC = 8
FIN = 500
FPAD = 512
HID = 32
NCLS = 40
PW = 34          # payload row width (bf16): 32 h + 1 invr + 1 pad
EPS_NORM = 1e-12
EPS_SM = 1e-16
SMAX = 320       # max gathered slots per indirect-DMA group


# ---------------------------------------------------------------- host prep

def _prep_graph(n, edge_index):
    """Degree-sort nodes, deal round-robin to cores, build padded slot tables."""
    src = np.asarray(edge_index[0], dtype=np.int64)
    dst = np.asarray(edge_index[1], dtype=np.int64)
    selfe = src == dst
    mself = np.bincount(dst[selfe], minlength=n)
    keep = ~selfe                               # self-loops handled on-chip
    src, dst = src[keep], dst[keep]
    e = src.shape[0]

    deg = np.bincount(dst, minlength=n)
    order = np.argsort(-deg, kind="stable")          # global rank -> node id
    rank = np.empty(n, dtype=np.int64)
    rank[order] = np.arange(n)

    nsh = -(-n // NC)                                 # nodes per core shard
    tiles = -(-nsh // 128)
    nloc = tiles * 128
    fb = nsh - (tiles - 1) * 128                      # real rows in last tile

    deg_sorted = deg[order]
    ks = []
    for t in range(tiles):
        w0 = t * 128 * NC
        k = int(deg_sorted[w0]) if w0 < n else 1
        ks.append(max(k, 1))
    ks = np.array(ks, dtype=np.int64)
    col0 = np.concatenate([[0], np.cumsum(ks)])       # slot column of tile t
    s_total = int(col0[-1])

    # group tiles: one indirect DMA per group, sum(k) <= SMAX
    groups = []  # (first_tile, ntiles, col_base, slots)
    t = 0
    while t < tiles:
        t0, s = t, 0
        while t < tiles and (t == t0 or s + ks[t] <= SMAX):
            s += ks[t]
            t += 1
        groups.append((t0, t - t0, int(col0[t0]), int(s)))

    # slot column for each edge
    r_dst = rank[dst]
    eorder = np.argsort(r_dst, kind="stable")
    rs = r_dst[eorder]
    ssrc = src[eorder]
    cum = np.concatenate([[0], np.cumsum(deg_sorted)])
    k_in_dst = np.arange(e, dtype=np.int64) - cum[rs]

    core = rs % NC
    local = rs // NC
    t_of = local // 128
    p_of = local % 128

    # table position (payload row) of each node: owner core block + local rank
    tpos = (rank % NC) * nloc + rank // NC
    ztab = nsh                                       # core0's first fake row

    offs = np.full((NC, 128, s_total), ztab, dtype=np.int32)
    offs[core, p_of, col0[t_of] + k_in_dst] = tpos[ssrc].astype(np.int32)

    # per-(core,p,t) degree / self-multiplicity for zadj + self-loop terms
    ranks = np.arange(n)
    c_a, l_a = ranks % NC, ranks // NC
    degarr = np.zeros((NC, 128, tiles), dtype=np.float64)
    degarr[c_a, l_a % 128, l_a // 128] = deg_sorted
    msl = np.zeros((NC, 128, tiles), dtype=np.float64)
    msl[c_a, l_a % 128, l_a // 128] = mself[order]
    padcnt = ks[None, None, :] - degarr              # fake rows: deg 0 -> k

    return dict(order=order, nsh=nsh, tiles=tiles, nloc=nloc, fb=fb,
                ks=[int(k) for k in ks], col0=[int(c) for c in col0],
                s_total=s_total, groups=groups, offs=offs,
                msl=msl, padcnt=padcnt)


# ---------------------------------------------------------------- device build

def _build(tiles, nloc, ks, col0, groups, s_total, stage=99):
    nc = bacc.Bacc("TRN2", target_bir_lowering=False, debug=False,
                   num_devices=NC)

    xT = nc.dram_tensor("xT", [FPAD, nloc], BF16, kind="ExternalInput")
    W1 = nc.dram_tensor("W1", [FPAD, HID], BF16, kind="ExternalInput")
    W2 = nc.dram_tensor("W2", [HID, NCLS], BF16, kind="ExternalInput")
    b2b = nc.dram_tensor("b2b", [128, NCLS], F32, kind="ExternalInput")
    bsc = nc.dram_tensor("bsc", [128, 8], F32, kind="ExternalInput")
    sadj = nc.dram_tensor("sadj", [128, 4 * tiles], F32, kind="ExternalInput")
    offs = nc.dram_tensor("offs", [128 * s_total], I32, kind="ExternalInput")
    out = nc.dram_tensor("out", [nloc, NCLS], F32, kind="ExternalOutput")

    kmax = max(ks)

    with tile.TileContext(nc) as tc:
        with tc.tile_pool(name="const", bufs=1) as cpool, \
             tc.tile_pool(name="persist", bufs=1) as ppool, \
             tc.tile_pool(name="xio", bufs=3) as xpool, \
             tc.tile_pool(name="gio", bufs=2) as gpool, \
             tc.tile_pool(name="work", bufs=6) as wpool, \
             tc.tile_pool(name="small", bufs=8) as spool, \
             tc.tile_pool(name="ps", bufs=2, space="PSUM") as pspool, \
             tc.tile_pool(name="dram", bufs=1, space="DRAM") as dpool:

            # ---- constants to SBUF
            w1_sb = cpool.tile([128, 4, HID], BF16)
            nc.sync.dma_start(w1_sb[:], W1.ap().rearrange("(c p) h -> p c h", p=128))
            w2_sb = cpool.tile([HID, NCLS], BF16)
            nc.sync.dma_start(w2_sb[:], W2.ap())
            b2_sb = cpool.tile([128, NCLS], F32)
            nc.sync.dma_start(b2_sb[:], b2b.ap())
            bsc_sb = cpool.tile([128, 8], F32)
            nc.sync.dma_start(bsc_sb[:], bsc.ap())
            sadj_sb = cpool.tile([128, 4 * tiles], F32)
            nc.sync.dma_start(sadj_sb[:], sadj.ap())
            off_sb = cpool.tile([128, s_total], I32)
            nc.sync.dma_start(off_sb[:],
                              offs.ap().rearrange("(p s) -> p s", s=s_total))
            ident = cpool.tile([128, 128], F32)
            make_identity(nc, ident[:])
            c_eps = cpool.tile([128, 1], F32)
            nc.gpsimd.memset(c_eps[:], EPS_NORM)
            c_neg1 = cpool.tile([128, 1], F32)
            nc.gpsimd.memset(c_neg1[:], -1.0)

            # ---- persistent node state: payload [h | invr | pad] per layer
            pd0 = ppool.tile([128, tiles, PW], BF16)
            pd1 = ppool.tile([128, tiles, PW], BF16)
            inv0 = ppool.tile([128, tiles], F32)      # 1/|h0| per node
            invb = ppool.tile([128, tiles], F32)      # beta2/|h1| per node

            pay1 = dpool.tile([nloc, PW], BF16)
            pay2 = dpool.tile([nloc, PW], BF16)
            tab1 = dpool.tile([NC * nloc, PW], BF16)
            tab2 = dpool.tile([NC * nloc, PW], BF16)

            xT_re = xT.ap().rearrange("(c p) n -> p c n", p=128)

            def payload_finish(pd, inv_ap, pay_dram, t):
                """ss = sum h^2; invr = 1/sqrt(ss+eps) -> inv col + payload."""
                sq = spool.tile([128, HID], BF16, tag="sq")
                ss = spool.tile([128, 1], F32, tag="ss")
                nc.scalar.activation(sq[:], pd[:, t, 0:HID], AF.Square,
                                     accum_out=ss[:])
                rt = spool.tile([128, 1], F32, tag="rt")
                nc.scalar.activation(rt[:], ss[:], AF.Sqrt, bias=c_eps[:])
                nc.vector.reciprocal(inv_ap[:, t:t + 1], rt[:])
                nc.vector.tensor_copy(pd[:, t, HID:HID + 1], inv_ap[:, t:t + 1])
                if t == tiles - 1:
                    # zero payload rows of fake (padding) nodes; bsc[:,2] is
                    # 1 for real partitions of the last tile, 0 for fake
                    nc.vector.tensor_scalar_mul(pd[:, t, :], pd[:, t, :],
                                                bsc_sb[:, 2:3])
                if t % 2 == 1:
                    nc.sync.dma_start(
                        pay_dram[(t - 1) * 128:(t + 1) * 128, :]
                        .rearrange("(a p) d -> p a d", p=128),
                        pd[:, t - 1:t + 1, :])
                elif t == tiles - 1:
                    nc.sync.dma_start(pay_dram[t * 128:(t + 1) * 128, :],
                                      pd[:, t, :])

            def dump_h(t, ap32):
                ot = spool.tile([128, NCLS], F32, tag="ot_dbg")
                nc.gpsimd.memset(ot[:], 0.0)
                nc.vector.tensor_copy(ot[:, 0:HID], ap32)
                nc.sync.dma_start(out.ap()[t * 128:(t + 1) * 128, :], ot[:])

            # ================= layer 1 (x @ W1 + b1 via ones-row) + payload
            for t2 in range(0, tiles, 4):
                npair = min(4, tiles - t2)
                xt = xpool.tile([128, 4, 512], BF16, tag="xt")
                nc.sync.dma_start(
                    xt[:, :, 0:npair * 128],
                    xT_re[:, :, t2 * 128:(t2 + npair) * 128])
                for dt_ in range(npair):
                    t = t2 + dt_
                    ps = pspool.tile([128, HID], F32, tag="ps1")
                    for c in range(4):
                        nc.tensor.matmul(ps[:],
                                         lhsT=xt[:, c, dt_ * 128:(dt_ + 1) * 128],
                                         rhs=w1_sb[:, c, :],
                                         start=(c == 0), stop=(c == 3))
                    nc.vector.tensor_relu(pd0[:, t, 0:HID], ps[:])
                    if stage == 1:
                        dump_h(t, pd0[:, t, 0:HID])
                        continue
                    payload_finish(pd0, inv0, pay1, t)

            if stage >= 2:
                nc.gpsimd.collective_compute(
                    "AllGather", ALU.bypass,
                    replica_groups=[list(range(NC))],
                    ins=[pay1.opt()], outs=[tab1.opt()])

            # ================= conv layer
            def conv(table, pd_src, scale_col, bias_arg, zadj_base, msw_base,
                     epilogue):
                for (t0, nt, cb, sg) in groups:
                    for t in range(t0, t0 + nt):
                        k = ks[t]
                        gbuf = gpool.tile([128, kmax * PW], BF16, tag="g")
                        for kk in range(k):
                            nc.gpsimd.indirect_dma_start(
                                out=gbuf[:, kk * PW:(kk + 1) * PW],
                                out_offset=None, in_=table[:],
                                in_offset=IndirectOffsetOnAxis(
                                    ap=off_sb[:, col0[t] + kk:col0[t] + kk + 1],
                                    axis=0))
                        g3 = gbuf[:, 0:k * PW].rearrange(
                            "p (k d) -> p k d", d=PW)

                        pl = wpool.tile([128, kmax * HID], BF16, tag="pl")
                        pl3 = pl[:, 0:k * HID].rearrange("p (k f) -> p k f",
                                                         f=HID)
                        dbc = pd_src[:, t, 0:HID].unsqueeze(1).to_broadcast(
                            [128, k, HID])
                        nc.vector.tensor_tensor(pl3, g3[:, :, 0:HID], dbc,
                                                ALU.mult)
                        llr = wpool.tile([128, kmax], F32, tag="llr")
                        nc.vector.tensor_reduce(llr[:, 0:k], pl3, AX.X, ALU.add)
                        # ll = llr * invr_src (strided lane-32 read)
                        ll = wpool.tile([128, kmax], F32, tag="ll")
                        nc.vector.tensor_tensor(
                            ll[:, 0:k], llr[:, 0:k],
                            g3[:, :, HID:HID + 1].rearrange("p k d -> p (k d)"),
                            ALU.mult)
                        # w = exp(scale*ll + bias), z = sum(w) fused
                        w = wpool.tile([128, kmax], BF16, tag="w")
                        z = spool.tile([128, 1], F32, tag="z")
                        nc.scalar.activation(w[:, 0:k], ll[:, 0:k], AF.Exp,
                                             bias=bias_arg,
                                             scale=scale_col(t),
                                             accum_out=z[:])
                        z2 = spool.tile([128, 1], F32, tag="z2")
                        nc.vector.tensor_scalar(
                            out=z2[:], in0=z[:],
                            scalar1=sadj_sb[:, zadj_base + t:zadj_base + t + 1],
                            scalar2=float(EPS_SM), op0=ALU.add, op1=ALU.add)
                        rz = spool.tile([128, 1], F32, tag="rz")
                        nc.vector.reciprocal(rz[:], z2[:])

                        pa = wpool.tile([128, kmax * HID], BF16, tag="pa")
                        pa3 = pa[:, 0:k * HID].rearrange("p (k f) -> p k f",
                                                         f=HID)
                        wbc = w[:, 0:k].unsqueeze(2).to_broadcast([128, k, HID])
                        nc.vector.tensor_tensor(pa3, g3[:, :, 0:HID], wbc,
                                                ALU.mult)
                        agg = spool.tile([128, HID], F32, tag="agg")
                        nc.vector.tensor_reduce(
                            agg[:],
                            pa[:, 0:k * HID].rearrange("p (k f) -> p f k",
                                                       f=HID),
                            AX.X, ALU.add)
                        # self-loop: agg += msl*wself * h_dst
                        hw = spool.tile([128, HID], F32, tag="hwself")
                        nc.vector.tensor_scalar_mul(
                            hw[:], pd_src[:, t, 0:HID],
                            sadj_sb[:, msw_base + t:msw_base + t + 1])
                        nc.vector.tensor_add(agg[:], agg[:], hw[:])
                        h_new = spool.tile([128, HID], F32, tag="hnew")
                        nc.vector.tensor_scalar_mul(h_new[:], agg[:], rz[:])
                        epilogue(t, h_new)

            def ep_conv1(t, h_new):
                nc.vector.tensor_copy(pd1[:, t, 0:HID], h_new[:])
                if stage == 2:
                    dump_h(t, h_new[:])
                    return
                payload_finish(pd1, invb, pay2, t)

            if stage >= 2:
                conv(tab1, pd0, lambda t: inv0[:, t:t + 1], c_neg1[:],
                     0, tiles, ep_conv1)

            if stage >= 3:
                # invb currently holds 1/|h1|; scale by beta2 for conv2's exp
                nc.vector.tensor_scalar_mul(invb[:], invb[:], bsc_sb[:, 0:1])

                nc.gpsimd.collective_compute(
                    "AllGather", ALU.bypass,
                    replica_groups=[list(range(NC))],
                    ins=[pay2.opt()], outs=[tab2.opt()])

            def ep_conv2(t, h_new):
                tp = pspool.tile([HID, 128], F32, tag="tp")
                nc.tensor.transpose(out=tp[:], in_=h_new[:], identity=ident[:])
                h2t = spool.tile([HID, 128], BF16, tag="h2t")
                nc.vector.tensor_copy(h2t[:], tp[:])
                ps2 = pspool.tile([128, NCLS], F32, tag="ps2")
                nc.tensor.matmul(ps2[:], lhsT=h2t[:], rhs=w2_sb[:],
                                 start=True, stop=True)
                ot = spool.tile([128, NCLS], F32, tag="ot")
                nc.vector.tensor_tensor(ot[:], ps2[:], b2_sb[:], ALU.add)
                nc.sync.dma_start(out.ap()[t * 128:(t + 1) * 128, :], ot[:])

            if stage >= 3:
                conv(tab2, pd1, lambda t: invb[:, t:t + 1], bsc_sb[:, 1:2],
                     2 * tiles, 3 * tiles, ep_conv2)

    nc.compile()
    return nc


# ---------------------------------------------------------------- entry point

def kernel(x, W1, b1, W2, b2, beta2, edge_index):
    x = np.asarray(x, dtype=np.float32)
    W1 = np.asarray(W1, dtype=np.float32)
    b1 = np.asarray(b1, dtype=np.float32)
    W2 = np.asarray(W2, dtype=np.float32)
    b2 = np.asarray(b2, dtype=np.float32)
    beta2 = float(np.asarray(beta2))
    edge_index = np.asarray(edge_index)

    n = x.shape[0]
    g = _prep_graph(n, edge_index)
    tiles, nloc, nsh, fb = g["tiles"], g["nloc"], g["nsh"], g["fb"]

    nc = _build(tiles, nloc, g["ks"], g["col0"], g["groups"], g["s_total"],
                stage=int(os.environ.get("AGNN_STAGE", "99")))

    w1p = np.zeros((FPAD, HID), dtype=ml_dtypes.bfloat16)
    w1p[:FIN] = W1.astype(ml_dtypes.bfloat16)
    w1p[FIN] = b1.astype(ml_dtypes.bfloat16)
    w2 = W2.astype(ml_dtypes.bfloat16)
    b2b = np.broadcast_to(b2, (128, NCLS)).copy().astype(np.float32)

    bscv = np.zeros((128, 8), dtype=np.float32)
    bscv[:, 0] = beta2
    bscv[:, 1] = -abs(beta2)
    bscv[:fb, 2] = 1.0                          # real-row mask for last tile

    wself1, wself2 = 1.0, float(np.exp(beta2 - abs(beta2)))
    ep1, ep2 = float(np.exp(-1.0)), float(np.exp(-abs(beta2)))
    sadjv = np.zeros((NC, 128, 4 * tiles), dtype=np.float32)
    sadjv[:, :, 0:tiles] = g["msl"] * wself1 - g["padcnt"] * ep1      # zadj1
    sadjv[:, :, tiles:2 * tiles] = g["msl"] * wself1                  # msw1
    sadjv[:, :, 2 * tiles:3 * tiles] = g["msl"] * wself2 - g["padcnt"] * ep2
    sadjv[:, :, 3 * tiles:4 * tiles] = g["msl"] * wself2              # msw2

    order = g["order"]
    xbf = x.astype(ml_dtypes.bfloat16)
    in_maps = []
    for c in range(NC):
        ids = order[c::NC]
        xs = np.zeros((FPAD, nloc), dtype=ml_dtypes.bfloat16)
        xs[:FIN, :len(ids)] = xbf[ids].T
        xs[FIN, :] = 1.0
        in_maps.append({
            "xT": xs, "W1": w1p, "W2": w2, "b2b": b2b, "bsc": bscv,
            "sadj": sadjv[c], "offs": g["offs"][c].reshape(-1),
        })

    trace = os.environ.get("AGNN_TRACE", "") == "1"
    kwargs = {}
    if trace:
        _enable_ntff_hook()
        import tempfile
        base = os.environ.get("AGNN_TRACE_DIR", "/tmp/agnn_traces")
        os.makedirs(base, exist_ok=True)
        kwargs = dict(trace=True, tmpdir=tempfile.mkdtemp(dir=base))
    res = run_bass_kernel_spmd(nc, in_maps, core_ids=list(range(NC)), **kwargs)
    if trace:
        print("AGNN exec_time_ns:", res.exec_time_ns)
        kernel._last_exec_time_ns = res.exec_time_ns

    out_full = np.empty((n, NCLS), dtype=np.float32)
    for c in range(NC):
        ids = order[c::NC]
        out_full[ids] = res.results[c]["out"][:len(ids)]
    return out_full


def _enable_ntff_hook():
    import antenv
    if "antenv.axon_hooks" not in sys.modules:
        mod = types.ModuleType("antenv.axon_hooks")
        _h = [None]
        mod.set_axon_ntff_profile_hook = lambda v: _h.__setitem__(0, v)
        mod.get_axon_ntff_profile_hook = lambda: _h[0]
        sys.modules["antenv.axon_hooks"] = mod
        antenv.axon_hooks = mod
    import concourse.bass_utils as bu
    bu.upload_artifacts = lambda d: d
    from trn_agent_boot.trn_boot import _ntff_profile_via_ctypes
    sys.modules["antenv.axon_hooks"].set_axon_ntff_profile_hook(
        _ntff_profile_via_ctypes("/opt/axon/libaxon_pjrt.so"))
